# revision 11
# baseline (speedup 1.0000x reference)
"""EntityCrossAttention Trainium2 kernel (bf16-streamed, PE-tiled).

Reference computation (per batch b):
    E = noun_feats[class_ids[b]];  Q = X @ Wq.T + bq;  K,V = E proj
    S = Q @ K.T / sqrt(D);  attn = softmax(S)
    wa = attn * w;  wa /= wa.sum(-1) + 1e-6;  out = wa @ V

Algebra:
  * S = X @ M / sqrt(D) + bias with M = Wq.T @ K.T  [D, N] host-precomputed
    (O(B*N*D^2) total, T-independent).
  * Since sum_n attn_n == 1 exactly, the noun reweighting + renorm is a
    plain softmax with ln(w + 1e-6) folded into the per-(b,n) exp bias
    (up to a negligible 1e-6 * attn * V numerator perturbation):
        out = (e @ V) / (e @ 1),   e = exp(S/sqrt(D) + eb)

Both HBM streams are bf16 (X in, out back): 8 MiB in + 8 MiB out per core.
The steady-state period is DMA-bound at ~410 GB/s shared read+write
(~2.6 us per 512-row group). Engine budget per group keeps every other
engine under that:
    PE    : 4 score MMs + 4 den MMs + 4 out MMs + 1 heater  (~2.3 us warm)
    ACT   : exp + 2 PSUM->SBUF scaled copies                (~2.2 us)
    DVE   : 2 scaled copies + reciprocal                    (~1.6 us)
    Sync  : X load + out store triggers, batched 2 groups per trigger
            (~0.6 us + waits; unbatched the two ~600 ns triggers plus
            their dependency waits saturate Sync and head-of-line block
            the load stream)
The heater matmul (operands: resident SBUF tiles, result overwritten by
the group's real scores via start=True) fills the PE's DMA-wait gap.
HAM discipline: the PE clock gate re-throttles 2.4->1.2 GHz after ~1 us
of PE idle (MID window) and only re-warms after ~3.4 us of gapless PE
activity, so every >1 us PE bubble costs ~10 us of half-clock matmuls;
the warm-up burst, heaters, trigger batching, and PSUM slack all exist
to keep PE bubbles under that threshold.
Sharding: data-parallel over B: 8 cores x 2 batches.
"""

import numpy as np

B, T, D, C, N = 16, 4096, 512, 14, 32
N_CORES = 8
B_PC = B // N_CORES          # batches per core
ROWS_PC = B_PC * T           # 8192
RT = 128                     # row subtile
GR = 512                     # rows per group
G_PC = ROWS_PC // GR         # 16 groups per core
KC = D // 128                # 4 contraction chunks
NB = GR // RT                # 4 row-subtile bands
GW = KC * GR                 # 2048 columns per group in x / out layouts
SCALE = float(D) ** -0.5

_compiled = None


def _build():
    import concourse.bacc as bacc
    import concourse.tile as tile
    import concourse.mybir as mybir

    f32 = mybir.dt.float32
    bf16 = mybir.dt.bfloat16
    Exp = mybir.ActivationFunctionType.Exp
    Copy = mybir.ActivationFunctionType.Copy

    nc = bacc.Bacc("TRN2", debug=False)
    # x[p, gg*GW + k*GR + r] = X[d=k*128+p, row gg*GR+r]
    # (per-partition contiguous: one 2-group load = 128 x 8 KiB descriptors)
    x = nc.dram_tensor("x", [128, G_PC * GW], bf16, kind="ExternalInput").ap()
    m = nc.dram_tensor("m", [128, B_PC * KC * N], bf16, kind="ExternalInput").ap()
    # v[n, b*D + d] = V[b, n, d]
    v = nc.dram_tensor("v", [N, B_PC * D], bf16, kind="ExternalInput").ap()
    # eb[n, b] = ebias[b, n]
    eb = nc.dram_tensor("eb", [N, B_PC], f32, kind="ExternalInput").ap()
    ones = nc.dram_tensor("ones", [N, 2], bf16, kind="ExternalInput").ap()
    # out[p, gg*GW + a*D + d] = out_row[gg*GR + a*RT + p, d]
    out = nc.dram_tensor("out", [128, G_PC * GW], bf16,
                         kind="ExternalOutput").ap()

    with tile.TileContext(nc) as tc:
        with (
            tc.tile_pool(name="const", bufs=1) as cpool,
            tc.tile_pool(name="xin", bufs=7) as xpool,
            tc.tile_pool(name="et", bufs=3) as epool,
            tc.tile_pool(name="rcp", bufs=3) as rcpool,
            tc.tile_pool(name="res", bufs=3) as rpool,
            tc.tile_pool(name="ps_sc", bufs=2, space="PSUM") as ps_sc,
            tc.tile_pool(name="ps_den", bufs=1, space="PSUM") as ps_den,
            tc.tile_pool(name="ps_o", bufs=5, space="PSUM") as ps_o,
        ):
            # Constants lead the Sync ring (~0.14 MB, ~0.4 us) ahead of the
            # X stream. Everything DMA rides Sync; ACT/DVE only compute.
            m_sb = cpool.tile([128, B_PC * KC * N], bf16)
            nc.sync.dma_start(m_sb[:, :], m[:, :])
            eb_sb = cpool.tile([N, B_PC], f32)
            nc.sync.dma_start(eb_sb[:, :], eb[:, :])
            ones_sb = cpool.tile([N, 2], bf16)
            nc.sync.dma_start(ones_sb[:, :], ones[:, :])
            v_sb = cpool.tile([N, B_PC * D], bf16)
            nc.sync.dma_start(v_sb[:, :], v[:, :])

            # HAM warm-up: a gapless burst of dummy matmuls while the first
            # X tiles stream in, so the PE clock is at 2.4 GHz from the
            # first real group onward. Operands come from a memset tile so
            # the burst starts immediately (no DMA dependency) and the PE
            # start stays ~2 groups behind the load stream.
            warm_sb = cpool.tile([128, 256], bf16)
            nc.vector.memset(warm_sb[:, :], 0.0)
            sc_warm = ps_sc.tile([N, GR], f32, tag="sc_ps")
            for _ in range(20):
                nc.tensor.matmul(
                    sc_warm[:, 0:256],
                    warm_sb[:, 0:N],
                    warm_sb[:, :],
                    start=True, stop=True,
                )

            o2_map = {}

            def emit_outs(e2, b, gg):
                pair, j = divmod(gg, 2)
                if j == 0:
                    o2_map[pair] = rpool.tile([RT, 2 * NB * D], bf16,
                                              tag="o_sb", name="o2_sb")
                o2 = o2_map[pair]
                obase = j * NB * D

                den_ps = ps_den.tile([RT, 2 * NB], f32)
                for a in range(NB):
                    nc.tensor.matmul(
                        den_ps[:, 2 * a : 2 * a + 2],
                        e2[:, a * RT : (a + 1) * RT],
                        ones_sb[0:N, :],
                        start=True, stop=True,
                    )
                rc_sb = rcpool.tile([RT, NB], f32)
                nc.vector.reciprocal(rc_sb[:, :], den_ps[:, ::2])

                # 4 row-tiled out matmuls (K=32 bands, concurrent)
                o_ps = []
                for a in range(NB):
                    o_ps_a = ps_o.tile([RT, D], f32, tag="o_ps")
                    o_ps.append(o_ps_a)
                    nc.tensor.matmul(
                        o_ps_a[:, :],
                        e2[:, a * RT : (a + 1) * RT],
                        v_sb[0:N, b * D : (b + 1) * D],
                        start=True, stop=True,
                    )
                tail = gg >= G_PC - 4
                for a in range(NB):
                    dst = o2[:, obase + a * D : obase + (a + 1) * D]
                    rc = rc_sb[:, a : a + 1]
                    if a % 2 == 0:
                        nc.scalar.activation(dst, o_ps[a][:, :], Copy, scale=rc)
                    else:
                        nc.vector.tensor_scalar_mul(dst, o_ps[a][:, :], rc)
                    if tail and a % 2 == 1:
                        # last two groups: ship half-group chunks on the
                        # (by now idle, low-latency) Sync queue as soon as
                        # each half's copies land, to shorten the store tail
                        # after the final matmuls
                        h = a // 2
                        nc.sync.dma_start(
                            out[:, gg * GW + h * 2 * D : gg * GW + (h + 1) * 2 * D],
                            o2[:, obase + h * 2 * D : obase + (h + 1) * 2 * D],
                        )
                if not tail:
                    # Steady-state stores on the GPSIMD ring: its ~1-3 us
                    # dispatch latency is harmless here (the o2 WAR slack is
                    # ~3 pairs) and it keeps Sync free for the load stream
                    # and ACT/DVE free for compute.
                    nc.gpsimd.dma_start(
                        out[:, gg * GW : (gg + 1) * GW],
                        o2[:, obase : obase + NB * D],
                    )

            prev = None
            prev_xv = None
            for gg in range(G_PC):
                b = gg // (T // GR)
                # Per-group loads: finer arrival granularity than pair
                # batching, so a ramp-phase PE never waits >1 group for X
                # (a >~1 us PE bubble trips the HAM MID re-throttle).
                xv = xpool.tile([128, GW], bf16, tag="x_sb", name="x_sb")
                nc.sync.dma_start(xv[:, :], x[:, gg * GW : (gg + 1) * GW])

                sc_ps = ps_sc.tile([N, GR], f32, tag="sc_ps")
                # Heater: one dummy matmul on resident operands, issued
                # ahead of the scores so the PE has work while waiting
                # for this group's X DMA. Result is discarded (the real
                # scores start=True reset the PSUM accumulation group).
                if prev_xv is None:
                    nc.tensor.matmul(
                        sc_ps[:, 0:256],
                        m_sb[:, 0:N],
                        m_sb[:, 0:256],
                        start=True, stop=True,
                    )
                else:
                    nc.tensor.matmul(
                        sc_ps[:, :],
                        m_sb[:, 0:N],
                        prev_xv[:, 0:GR],
                        start=True, stop=True,
                    )

                for k in range(KC):
                    nc.tensor.matmul(
                        sc_ps[:, :],
                        m_sb[:, (b * KC + k) * N : (b * KC + k + 1) * N],
                        xv[:, k * GR : (k + 1) * GR],
                        start=(k == 0),
                        stop=(k == KC - 1),
                    )
                e_sb = epool.tile([N, GR], bf16, tag="e_sb")
                nc.scalar.activation(
                    e_sb[:, :], sc_ps[:, :], Exp,
                    bias=eb_sb[0:N, b : b + 1], scale=SCALE,
                )

                # previous group's den/out matmuls fill the PE while
                # this group's exp runs on ACT
                if prev is not None:
                    emit_outs(*prev)
                prev = (e_sb, b, gg)
                prev_xv = xv
            emit_outs(*prev)

    nc.compile()
    return nc


def _get_compiled():
    global _compiled
    if _compiled is None:
        _compiled = _build()
    return _compiled


def kernel(
    visual_feat, noun_feats, class_ids, noun_weights,
    Wq, bq, Wk, bk, Wv, bv,
):
    import ml_dtypes
    from concourse.bass_utils import run_bass_kernel_spmd

    bfloat16 = ml_dtypes.bfloat16
    visual_feat = np.asarray(visual_feat, dtype=np.float32)
    noun_feats = np.asarray(noun_feats, dtype=np.float32)
    class_ids = np.asarray(class_ids)
    noun_weights = np.asarray(noun_weights, dtype=np.float32)
    Wq, bq = np.asarray(Wq, np.float32), np.asarray(bq, np.float32)
    Wk, bk = np.asarray(Wk, np.float32), np.asarray(bk, np.float32)
    Wv, bv = np.asarray(Wv, np.float32), np.asarray(bv, np.float32)

    # Host precompute of per-batch constants (all T-independent).
    E = noun_feats[class_ids]                       # [B, N, D]
    W = noun_weights[class_ids]                     # [B, N]
    Kb = E @ Wk.T + bk                              # [B, N, D]
    Vb = E @ Wv.T + bv                              # [B, N, D]
    M = np.einsum("jd,bnj->bdn", Wq, Kb)            # [B, D, N] = Wq.T @ Kb.T
    # exp bias: bq-projection term + ln(w + 1e-6) reweighting fold
    ebias = (Kb @ bq) * SCALE + np.log(W + 1e-6)    # [B, N]

    nc = _get_compiled()

    in_maps = []
    for c in range(N_CORES):
        s = slice(c * B_PC, (c + 1) * B_PC)
        m_c = np.ascontiguousarray(
            M[s].reshape(B_PC, KC, 128, N).transpose(2, 0, 1, 3).reshape(128, -1)
        ).astype(bfloat16)
        # x[p, gg*GW + k*GR + r] = Xt[k*128+p, gg*GR+r]
        xt_c = visual_feat[s].reshape(ROWS_PC, D).T.astype(bfloat16)
        x_c = np.ascontiguousarray(
            xt_c.reshape(KC, 128, G_PC, GR).transpose(1, 2, 0, 3)
        ).reshape(128, G_PC * GW)
        v_c = Vb[s].transpose(1, 0, 2).reshape(N, B_PC * D)
        eb_c = ebias[s].T                           # [N, B_PC]
        in_maps.append(
            {
                "x": x_c,
                "m": m_c,
                "v": np.ascontiguousarray(v_c).astype(bfloat16),
                "eb": np.ascontiguousarray(eb_c),
                "ones": np.ones((N, 2), np.float32).astype(bfloat16),
            }
        )

    global _last_in_maps
    _last_in_maps = in_maps
    res = run_bass_kernel_spmd(nc, in_maps, list(range(N_CORES)))
    out = np.empty((B, T, D), dtype=np.float32)
    for c in range(N_CORES):
        # out[p, gg*GW + a*D + d] -> rows
        o = res.results[c]["out"].reshape(128, G_PC, NB, D)
        o = o.transpose(1, 2, 0, 3).reshape(B_PC, T, D).astype(np.float32)
        out[c * B_PC : (c + 1) * B_PC] = o
    return out


# revision 12
# speedup vs baseline: 1.4780x; 1.4780x over previous
"""EntityCrossAttention Trainium2 kernel (bf16-streamed, PE-tiled).

Reference computation (per batch b):
    E = noun_feats[class_ids[b]];  Q = X @ Wq.T + bq;  K,V = E proj
    S = Q @ K.T / sqrt(D);  attn = softmax(S)
    wa = attn * w;  wa /= wa.sum(-1) + 1e-6;  out = wa @ V

Algebra:
  * S = X @ M / sqrt(D) + bias with M = Wq.T @ K.T  [D, N] host-precomputed
    (O(B*N*D^2) total, T-independent).
  * Since sum_n attn_n == 1 exactly, the noun reweighting + renorm is a
    plain softmax with ln(w + 1e-6) folded into the per-(b,n) exp bias
    (up to a negligible 1e-6 * attn * V numerator perturbation):
        out = (e @ V) / (e @ 1),   e = exp(S/sqrt(D) + eb)

Both HBM streams are bf16 (X in, out back): 8 MiB in + 8 MiB out per core.
The steady-state period is DMA-bound at ~410 GB/s shared read+write
(~2.6 us per 512-row group). Engine budget per group keeps every other
engine under that:
    PE    : 4 score MMs + 4 den MMs + 4 out MMs + 1 heater  (~2.3 us warm)
    ACT   : exp + 2 PSUM->SBUF scaled copies                (~2.2 us)
    DVE   : 2 scaled copies + reciprocal                    (~1.6 us)
    Sync  : X load + out store triggers, batched 2 groups per trigger
            (~0.6 us + waits; unbatched the two ~600 ns triggers plus
            their dependency waits saturate Sync and head-of-line block
            the load stream)
The heater matmul (operands: resident SBUF tiles, result overwritten by
the group's real scores via start=True) fills the PE's DMA-wait gap.
HAM discipline: the PE clock gate re-throttles 2.4->1.2 GHz after ~1 us
of PE idle (MID window) and only re-warms after ~3.4 us of gapless PE
activity, so every >1 us PE bubble costs ~10 us of half-clock matmuls;
the warm-up burst, heaters, trigger batching, and PSUM slack all exist
to keep PE bubbles under that threshold.
Sharding: data-parallel over B: 8 cores x 2 batches.
"""

import numpy as np

B, T, D, C, N = 16, 4096, 512, 14, 32
N_CORES = 8
B_PC = B // N_CORES          # batches per core
ROWS_PC = B_PC * T           # 8192
RT = 128                     # row subtile
GR = 512                     # rows per group
G_PC = ROWS_PC // GR         # 16 groups per core
KC = D // 128                # 4 contraction chunks
NB = GR // RT                # 4 row-subtile bands
GW = KC * GR                 # 2048 columns per group in x / out layouts
SCALE = float(D) ** -0.5

_compiled = None


def _build():
    import concourse.bacc as bacc
    import concourse.tile as tile
    import concourse.mybir as mybir

    f32 = mybir.dt.float32
    bf16 = mybir.dt.bfloat16
    Exp = mybir.ActivationFunctionType.Exp
    Copy = mybir.ActivationFunctionType.Copy

    nc = bacc.Bacc("TRN2", debug=False)
    # x[p, gg*GW + k*GR + r] = X[d=k*128+p, row gg*GR+r]
    # (per-partition contiguous: one 2-group load = 128 x 8 KiB descriptors)
    x = nc.dram_tensor("x", [128, G_PC * GW], bf16, kind="ExternalInput").ap()
    m = nc.dram_tensor("m", [128, B_PC * KC * N], bf16, kind="ExternalInput").ap()
    # v[n, b*D + d] = V[b, n, d]
    v = nc.dram_tensor("v", [N, B_PC * D], bf16, kind="ExternalInput").ap()
    # eb[n, b] = ebias[b, n]
    eb = nc.dram_tensor("eb", [N, B_PC], f32, kind="ExternalInput").ap()
    ones = nc.dram_tensor("ones", [N, 2], bf16, kind="ExternalInput").ap()
    # out[p, gg*GW + a*D + d] = out_row[gg*GR + a*RT + p, d]
    out = nc.dram_tensor("out", [128, G_PC * GW], bf16,
                         kind="ExternalOutput").ap()

    with tile.TileContext(nc) as tc:
        with (
            tc.tile_pool(name="const", bufs=1) as cpool,
            tc.tile_pool(name="xin", bufs=6) as xpool,
            tc.tile_pool(name="et", bufs=3) as epool,
            tc.tile_pool(name="rcp", bufs=3) as rcpool,
            tc.tile_pool(name="res", bufs=3) as rpool,
            tc.tile_pool(name="ps_sc", bufs=2, space="PSUM") as ps_sc,
            tc.tile_pool(name="ps_den", bufs=1, space="PSUM") as ps_den,
            tc.tile_pool(name="ps_o", bufs=5, space="PSUM") as ps_o,
        ):
            # Constants lead the Sync ring (~0.14 MB, ~0.4 us) ahead of the
            # X stream. Everything DMA rides Sync; ACT/DVE only compute.
            m_sb = cpool.tile([128, B_PC * KC * N], bf16)
            nc.sync.dma_start(m_sb[:, :], m[:, :])
            eb_sb = cpool.tile([N, B_PC], f32)
            nc.sync.dma_start(eb_sb[:, :], eb[:, :])
            ones_sb = cpool.tile([N, 2], bf16)
            nc.sync.dma_start(ones_sb[:, :], ones[:, :])
            v_sb = cpool.tile([N, B_PC * D], bf16)
            nc.sync.dma_start(v_sb[:, :], v[:, :])

            # HAM warm-up: a gapless burst of dummy matmuls while the first
            # X tiles stream in, so the PE clock is at 2.4 GHz from the
            # first real group onward. Gating the burst on m_sb arrival
            # self-aligns it with the DMA stream start, so the burst ends
            # about when the first X tiles (issued just after m on the same
            # queue) have landed plus ~2 groups of prefetch.
            sc_warm = ps_sc.tile([N, GR], f32, tag="sc_ps")
            for _ in range(22):
                nc.tensor.matmul(
                    sc_warm[:, 0:256],
                    m_sb[:, 0:N],
                    m_sb[:, 0:256],
                    start=True, stop=True,
                )

            o2_map = {}

            def emit_outs(e2, b, gg):
                pair, j = divmod(gg, 2)
                if j == 0:
                    o2_map[pair] = rpool.tile([RT, 2 * NB * D], bf16,
                                              tag="o_sb", name="o2_sb")
                o2 = o2_map[pair]
                obase = j * NB * D

                den_ps = ps_den.tile([RT, 2 * NB], f32)
                for a in range(NB):
                    nc.tensor.matmul(
                        den_ps[:, 2 * a : 2 * a + 2],
                        e2[:, a * RT : (a + 1) * RT],
                        ones_sb[0:N, :],
                        start=True, stop=True,
                    )
                rc_sb = rcpool.tile([RT, NB], f32)
                nc.vector.reciprocal(rc_sb[:, :], den_ps[:, ::2])

                # 4 row-tiled out matmuls (K=32 bands, concurrent)
                o_ps = []
                for a in range(NB):
                    o_ps_a = ps_o.tile([RT, D], f32, tag="o_ps")
                    o_ps.append(o_ps_a)
                    nc.tensor.matmul(
                        o_ps_a[:, :],
                        e2[:, a * RT : (a + 1) * RT],
                        v_sb[0:N, b * D : (b + 1) * D],
                        start=True, stop=True,
                    )
                tail = gg >= G_PC - 2
                for a in range(NB):
                    dst = o2[:, obase + a * D : obase + (a + 1) * D]
                    rc = rc_sb[:, a : a + 1]
                    if a % 2 == 0:
                        nc.scalar.activation(dst, o_ps[a][:, :], Copy, scale=rc)
                    else:
                        nc.vector.tensor_scalar_mul(dst, o_ps[a][:, :], rc)
                    if tail and a % 2 == 1:
                        # last two groups: ship half-group chunks on the
                        # (by now idle, low-latency) Sync queue as soon as
                        # each half's copies land, to shorten the store tail
                        # after the final matmuls
                        h = a // 2
                        nc.sync.dma_start(
                            out[:, gg * GW + h * 2 * D : gg * GW + (h + 1) * 2 * D],
                            o2[:, obase + h * 2 * D : obase + (h + 1) * 2 * D],
                        )
                if not tail:
                    # Steady-state stores on the GPSIMD ring: its ~1-3 us
                    # dispatch latency is harmless here (the o2 WAR slack is
                    # ~3 pairs) and it keeps Sync free for the load stream
                    # and ACT/DVE free for compute.
                    nc.gpsimd.dma_start(
                        out[:, gg * GW : (gg + 1) * GW],
                        o2[:, obase : obase + NB * D],
                    )

            prev = None
            prev_xv = None
            for gg in range(G_PC):
                b = gg // (T // GR)
                # Per-group loads: finer arrival granularity than pair
                # batching, so a ramp-phase PE never waits >1 group for X
                # (a >~1 us PE bubble trips the HAM MID re-throttle).
                xv = xpool.tile([128, GW], bf16, tag="x_sb", name="x_sb")
                nc.sync.dma_start(xv[:, :], x[:, gg * GW : (gg + 1) * GW])

                sc_ps = ps_sc.tile([N, GR], f32, tag="sc_ps")
                # Heater: one dummy matmul on resident operands, issued
                # ahead of the scores so the PE has work while waiting
                # for this group's X DMA. Result is discarded (the real
                # scores start=True reset the PSUM accumulation group).
                if prev_xv is None:
                    nc.tensor.matmul(
                        sc_ps[:, 0:256],
                        m_sb[:, 0:N],
                        m_sb[:, 0:256],
                        start=True, stop=True,
                    )
                else:
                    nc.tensor.matmul(
                        sc_ps[:, :],
                        m_sb[:, 0:N],
                        prev_xv[:, 0:GR],
                        start=True, stop=True,
                    )

                for k in range(KC):
                    nc.tensor.matmul(
                        sc_ps[:, :],
                        m_sb[:, (b * KC + k) * N : (b * KC + k + 1) * N],
                        xv[:, k * GR : (k + 1) * GR],
                        start=(k == 0),
                        stop=(k == KC - 1),
                    )
                e_sb = epool.tile([N, GR], bf16, tag="e_sb")
                nc.scalar.activation(
                    e_sb[:, :], sc_ps[:, :], Exp,
                    bias=eb_sb[0:N, b : b + 1], scale=SCALE,
                )

                # previous group's den/out matmuls fill the PE while
                # this group's exp runs on ACT
                if prev is not None:
                    emit_outs(*prev)
                prev = (e_sb, b, gg)
                prev_xv = xv
            emit_outs(*prev)

    nc.compile()
    return nc


def _get_compiled():
    global _compiled
    if _compiled is None:
        _compiled = _build()
    return _compiled


def kernel(
    visual_feat, noun_feats, class_ids, noun_weights,
    Wq, bq, Wk, bk, Wv, bv,
):
    import ml_dtypes
    from concourse.bass_utils import run_bass_kernel_spmd

    bfloat16 = ml_dtypes.bfloat16
    visual_feat = np.asarray(visual_feat, dtype=np.float32)
    noun_feats = np.asarray(noun_feats, dtype=np.float32)
    class_ids = np.asarray(class_ids)
    noun_weights = np.asarray(noun_weights, dtype=np.float32)
    Wq, bq = np.asarray(Wq, np.float32), np.asarray(bq, np.float32)
    Wk, bk = np.asarray(Wk, np.float32), np.asarray(bk, np.float32)
    Wv, bv = np.asarray(Wv, np.float32), np.asarray(bv, np.float32)

    # Host precompute of per-batch constants (all T-independent).
    E = noun_feats[class_ids]                       # [B, N, D]
    W = noun_weights[class_ids]                     # [B, N]
    Kb = E @ Wk.T + bk                              # [B, N, D]
    Vb = E @ Wv.T + bv                              # [B, N, D]
    M = np.einsum("jd,bnj->bdn", Wq, Kb)            # [B, D, N] = Wq.T @ Kb.T
    # exp bias: bq-projection term + ln(w + 1e-6) reweighting fold
    ebias = (Kb @ bq) * SCALE + np.log(W + 1e-6)    # [B, N]

    nc = _get_compiled()

    in_maps = []
    for c in range(N_CORES):
        s = slice(c * B_PC, (c + 1) * B_PC)
        m_c = np.ascontiguousarray(
            M[s].reshape(B_PC, KC, 128, N).transpose(2, 0, 1, 3).reshape(128, -1)
        ).astype(bfloat16)
        # x[p, gg*GW + k*GR + r] = Xt[k*128+p, gg*GR+r]
        xt_c = visual_feat[s].reshape(ROWS_PC, D).T.astype(bfloat16)
        x_c = np.ascontiguousarray(
            xt_c.reshape(KC, 128, G_PC, GR).transpose(1, 2, 0, 3)
        ).reshape(128, G_PC * GW)
        v_c = Vb[s].transpose(1, 0, 2).reshape(N, B_PC * D)
        eb_c = ebias[s].T                           # [N, B_PC]
        in_maps.append(
            {
                "x": x_c,
                "m": m_c,
                "v": np.ascontiguousarray(v_c).astype(bfloat16),
                "eb": np.ascontiguousarray(eb_c),
                "ones": np.ones((N, 2), np.float32).astype(bfloat16),
            }
        )

    global _last_in_maps
    _last_in_maps = in_maps
    res = run_bass_kernel_spmd(nc, in_maps, list(range(N_CORES)))
    out = np.empty((B, T, D), dtype=np.float32)
    for c in range(N_CORES):
        # out[p, gg*GW + a*D + d] -> rows
        o = res.results[c]["out"].reshape(128, G_PC, NB, D)
        o = o.transpose(1, 2, 0, 3).reshape(B_PC, T, D).astype(np.float32)
        out[c * B_PC : (c + 1) * B_PC] = o
    return out
